# revision 19
# baseline (speedup 1.0000x reference)
"""Local causal (sliding-window) attention on 8 Trainium2 NeuronCores.

Sequence-parallel: each core owns 512 consecutive query tokens of one batch
element (cores 0-3 -> batch 0, 4-7 -> batch 1) plus a 128-token halo whose
k/v are recomputed locally, so no inter-core communication is needed.

All matmuls run in bf16 (full PE rate at any moving size, half the HBM
bytes of fp32); PSUM accumulates fp32. Every input is host-prebaked into
the exact SBUF layout so each tensor arrives in a few fully contiguous
DMAs, issued in consumption order with dependency-gated staggering so the
round-robin DMA engine doesn't dilute early transfers with late weights.

The emission schedule software-pipelines engines: st0's attention pairs are
interleaved with the remaining projection groups (q-cb1, k-cb3, v halves),
st0's output projection fills st1's attention gaps, and denominators are
processed in sub-batches (batched SBUF->SBUF DMA to a [2n,256] tile,
reciprocal_approx_fast, casting DMA back, GpSimd partition-broadcasts) so
their serial chain hides under PE work.
"""

import sys

sys.path.insert(0, "/opt/trn_rl_repo")
import numpy as np
import ml_dtypes

BF16 = ml_dtypes.bfloat16

B, S, D = 2, 2048, 1024
H, DH = 16, 64
WINDOW = 128
NCORES = 8
SLOC = 512
HALO = 128
TLOC = SLOC + HALO
NST = 2
CPB = NCORES // B

_cached = {}


def _build():
    import concourse.bacc as bacc
    import concourse.mybir as mybir
    import concourse.tile as tile

    f32 = mybir.dt.float32
    bf16 = mybir.dt.bfloat16
    AF = mybir.ActivationFunctionType

    nc = bacc.Bacc(None)
    CSTW = NST * 512 + 16 + 128   # masks | ones16 | sel[2,128]
    xt_d = nc.declare_dram_parameter("xt", [128, 8 * TLOC], bf16, isOutput=False)
    wq_d = nc.declare_dram_parameter("wq", [128, 6 * 4096], bf16, isOutput=False)
    wo_d = nc.declare_dram_parameter("wo", [128, 8192], bf16, isOutput=False)
    cst_d = nc.declare_dram_parameter("cst", [128, CSTW], bf16, isOutput=False)
    out_d = nc.declare_dram_parameter("out", [128, NST * 2048], bf16, isOutput=True)

    with tile.TileContext(nc) as tc:
        with (
            tc.tile_pool(name="sb", bufs=1) as sb,
            tc.tile_pool(name="pjps", bufs=1, space="PSUM") as pjps,
            tc.tile_pool(name="scps", bufs=1, space="PSUM") as scps,
            tc.tile_pool(name="avps", bufs=1, space="PSUM") as avps,
        ):
            # ---- head DMAs, consumption-ordered. wq0 is m-major (4 chunks
            # so the first q-group gates on only 0.25MB of weights); xt in 2
            # halves so the first k-accumulation starts after half the x.
            wq0m = [sb.tile([128, 1024], bf16, tag=f"wq0m{m}", name=f"wq0m{m}")
                    for m in range(4)]
            nc.sync.dma_start(out=wq0m[0][:], in_=wq_d[:, 0:1024])
            xta = sb.tile([128, 4 * TLOC], bf16, tag="xta", name="xta")
            nc.sync.dma_start(out=xta[:], in_=xt_d[:, 0:4 * TLOC])
            xtb = sb.tile([128, 4 * TLOC], bf16, tag="xtb", name="xtb")
            nc.sync.dma_start(out=xtb[:], in_=xt_d[:, 4 * TLOC:8 * TLOC])

            def xt_sl(k, c0, c1):
                t = xta if k < 4 else xtb
                kk = k % 4
                return t[:, kk * TLOC + c0:kk * TLOC + c1]

            for m in range(1, 4):
                nc.sync.dma_start(
                    out=wq0m[m][:], in_=wq_d[:, m * 1024:(m + 1) * 1024]
                )
            cst = sb.tile([128, CSTW], bf16, tag="cst", name="cst")
            nc.sync.dma_start(out=cst[:], in_=cst_d[:])
            wq2 = sb.tile([128, 4096], bf16, tag="wq2", name="wq2")
            nc.sync.dma_start(out=wq2[:], in_=wq_d[:, 2 * 4096:3 * 4096])
            # later weight tiles are declared now but DMA'd behind a tiny
            # WAW "gate" write that keys each transfer to pipeline progress,
            # so early transfers keep full DMA bandwidth.
            wq4 = sb.tile([128, 4096], bf16, tag="wq4", name="wq4")
            wq5 = sb.tile([128, 4096], bf16, tag="wq5", name="wq5")
            wq1 = sb.tile([128, 4096], bf16, tag="wq1", name="wq1")
            wq3 = sb.tile([128, 4096], bf16, tag="wq3", name="wq3")
            woA = sb.tile([128, 4096], bf16, tag="woA", name="woA")
            woB = sb.tile([128, 4096], bf16, tag="woB", name="woB")

            def gated_dma(dst, col0, key_ap):
                nc.vector.tensor_copy(dst[0:1, 0:8], key_ap)
                nc.sync.dma_start(out=dst[:], in_=wq_d[:, col0:col0 + 4096]
                                  if col0 < 6 * 4096 else wo_d[:, col0 - 6 * 4096:col0 - 6 * 4096 + 4096])

            msk = [cst[:, 0:512], cst[:, 512:1024]]
            ones_sb = cst[:, 1024:1040]
            sel_sb = cst[0:2, 1040:1168]   # [2,128]: row0 = p<64, row1 = p>=64

            qT = [sb.tile([128, SLOC], bf16, tag=f"qT{i}", name=f"qT{i}") for i in range(8)]
            kT = [sb.tile([128, TLOC], bf16, tag=f"kT{i}", name=f"kT{i}") for i in range(8)]
            vt = [sb.tile([128, 65 * H], bf16, tag=f"v{t}", name=f"v{t}") for t in range(5)]
            att = [[sb.tile([128, 256], bf16, tag=f"at{st}_{t}", name=f"at{st}_{t}")
                    for t in range(8)] for st in range(NST)]
            ot = [sb.tile([128, 2048], bf16, tag=f"ot{st}", name=f"ot{st}") for st in range(NST)]

            # ---- projection group emitters (thunk lists of single ops) ----
            def q_group(cb, m):
                ps = pjps.tile([128, 512], f32, tag="qk", bufs=2, name=f"psq{cb}_{m}")
                th = []
                for k in range(8):
                    def mm(k=k, ps=ps, cb=cb, m=m):
                        if cb == 0:
                            lhs = wq0m[m][:, k * 128:(k + 1) * 128]
                        else:
                            lhs = wq1[:, k * 512 + m * 128:k * 512 + (m + 1) * 128]
                        nc.tensor.matmul(
                            ps[:], lhs, xt_sl(k, HALO, TLOC),
                            start=(k == 0), stop=(k == 7),
                        )
                    th.append(mm)
                if cb == 0:
                    th.append(lambda ps=ps, m=m: nc.scalar.copy(qT[m][:], ps[:]))
                else:
                    th.append(lambda ps=ps, m=m: nc.vector.tensor_copy(qT[4 + m][:], ps[:]))
                return th

            def k_group(cb, m, n):
                w = wq2 if cb == 2 else wq3
                ps = pjps.tile([128, 320], f32, tag="qk", bufs=2, name=f"psk{cb}_{m}_{n}")
                th = []
                for k in range(8):
                    def mm(k=k, ps=ps, w=w, m=m, n=n):
                        nc.tensor.matmul(
                            ps[:], w[:, k * 512 + m * 128:k * 512 + (m + 1) * 128],
                            xt_sl(k, n * 320, (n + 1) * 320),
                            start=(k == 0), stop=(k == 7),
                        )
                    th.append(mm)
                def kcp(ps=ps, cb=cb, m=m, n=n):
                    dst = kT[(cb - 2) * 4 + m][:, n * 320:(n + 1) * 320]
                    if cb == 2:
                        nc.scalar.copy(dst, ps[:])
                    else:
                        nc.vector.tensor_copy(dst, ps[:])
                th.append(kcp)
                return th

            def v_group(t, half):
                w = wq4 if half == 0 else wq5
                ps = pjps.tile([128, 512], f32, tag="qk", bufs=2, name=f"psv{t}_{half}")
                th = []
                for k in range(8):
                    def mm(k=k, ps=ps, w=w, t=t):
                        nc.tensor.matmul(
                            ps[:], xt_sl(k, t * 128, (t + 1) * 128),
                            w[:, k * 512:(k + 1) * 512],
                            start=(k == 0), stop=(k == 7),
                        )
                    th.append(mm)

                def cp(ps=ps, t=t, half=half):
                    h0 = half * 8
                    dst = vt[t].rearrange("p (h c) -> p h c", c=65)[:, h0:h0 + 8, 0:64]
                    src2 = ps[:].rearrange("p (h c) -> p h c", c=64)
                    if half == 0 and t < 3:
                        nc.scalar.copy(dst, src2)
                    else:
                        nc.vector.tensor_copy(dst, src2)
                th.append(cp)
                return th

            def po2_group(st, g):
                q0 = st * 256
                po = pjps.tile([128, 512], f32, tag="qk", bufs=2, name=f"po{st}_{g}")
                th = []
                for half in range(2):
                    m = 2 * g + half
                    c0 = half * 256
                    for k in range(8):
                        def mm(k=k, po=po, m=m, c0=c0, st=st, q0=q0):
                            wo = woA if k < 4 else woB
                            kk = k % 4
                            nc.tensor.matmul(
                                po[:, c0:c0 + 256],
                                wo[:, kk * 1024 + m * 128:kk * 1024 + (m + 1) * 128],
                                att[st][k][:, :],
                                start=(k == 0), stop=(k == 7),
                                skip_group_check=True,
                            )
                        th.append(mm)
                def pcp(po=po, st=st, g=g):
                    dst = ot[st][:, g * 512:(g + 1) * 512]
                    if st == 0:
                        nc.vector.tensor_copy(dst, po[:])
                    else:
                        nc.scalar.copy(dst, po[:])
                th.append(pcp)
                return th

            # ---- attention emitters ----
            pend = {}

            def emit_qk(st, h):
                q0 = st * 256
                jb = st * 2
                t, poff = h // 2, (h % 2) * 64
                sc = scps.tile([128, 512], f32, tag="sc", bufs=2, name=f"sc{st}_{h}")
                nc.tensor.matmul(
                    sc[:, 256:512],
                    kT[t][poff:poff + 64, (jb + 1) * 128:(jb + 2) * 128],
                    qT[t][poff:poff + 64, q0:q0 + 256],
                    start=True, stop=False, skip_group_check=True,
                )
                nc.tensor.matmul(
                    sc[:, 0:128],
                    kT[t][poff:poff + 64, jb * 128:(jb + 1) * 128],
                    qT[t][poff:poff + 64, q0:q0 + 128],
                    start=True, stop=False, skip_group_check=True,
                )
                nc.tensor.matmul(
                    sc[:, 128:256],
                    kT[t][poff:poff + 64, (jb + 2) * 128:(jb + 3) * 128],
                    qT[t][poff:poff + 64, q0 + 128:q0 + 256],
                    start=True, stop=True, skip_group_check=True,
                )
                p = sb.tile([128, 512], bf16, tag="pp", bufs=8, name=f"p{st}_{h}")
                nc.scalar.activation(p[:], sc[:], AF.Exp, scale=0.125)
                eng = nc.gpsimd if st == 1 else nc.vector
                eng.tensor_mul(p[:], p[:], msk[st])
                pend[(st, h)] = p

            def emit_av_pair(st, j, scat_b, pair_in_b, cast_eng):
                jb = st * 2
                t = j
                p0, p1 = pend.pop((st, 2 * j)), pend.pop((st, 2 * j + 1))
                av = avps.tile([65, 512], f32, tag="av", bufs=2, name=f"av{st}_{j}")
                for half, p in ((0, p0), (1, p1)):
                    c0 = half * 256
                    h = 2 * j + half
                    nc.tensor.matmul(
                        av[:, c0:c0 + 256], vt[jb + 1][:, h * 65:h * 65 + 65],
                        p[:, 256:512],
                        start=True, stop=False, skip_group_check=True,
                    )
                    nc.tensor.matmul(
                        av[:, c0:c0 + 128], vt[jb][:, h * 65:h * 65 + 65],
                        p[:, 0:128],
                        start=False, stop=False, skip_group_check=True,
                    )
                    nc.tensor.matmul(
                        av[:, c0 + 128:c0 + 256], vt[jb + 2][:, h * 65:h * 65 + 65],
                        p[:, 128:256],
                        start=False, stop=True, skip_group_check=True,
                    )
                dstv = scat_b[0:1, :].rearrange("p (h r) -> p h r", h=2)[
                    :, :, pair_in_b * 256:(pair_in_b + 1) * 256]
                nc.scalar.copy(dstv, av[64:65, :].rearrange("p (h c) -> p h c", h=2))
                if cast_eng == "s":
                    nc.scalar.copy(att[st][t][0:64, :], av[0:64, 0:256])
                    nc.scalar.copy(att[st][t][64:128, :], av[0:64, 256:512])
                else:
                    nc.vector.tensor_copy(att[st][t][0:64, :], av[0:64, 0:256])
                    nc.vector.tensor_copy(att[st][t][64:128, :], av[0:64, 256:512])

            def den_start(st, pairs, scat_b):
                n = len(pairs)
                s_b = sb.tile([2, n * 256], f32, tag="s_b", bufs=3, name=f"s{st}_{pairs[0]}")
                nc.sync.dma_start(out=s_b[:], in_=scat_b[0:1, :])
                r_b = sb.tile([2, n * 256], f32, tag="r_b", bufs=3, name=f"r{st}_{pairs[0]}")
                nc.vector.reciprocal_approx_fast(out=r_b[:], in_=s_b[:])
                rb16 = sb.tile([2, n * 256], bf16, tag="rb16", bufs=3, name=f"rb16_{st}_{pairs[0]}")
                nc.vector.tensor_copy(rb16[:], r_b[:])
                return rb16

            def den_finish(st, pairs, rb16):
                rb2 = None
                for i, j in enumerate(pairs):
                    if i % 2 == 0:
                        rb2 = scps.tile([128, 512], f32, tag="rb2", bufs=2,
                                        name=f"rb2_{st}_{j}")
                    c0 = (i % 2) * 256
                    nc.tensor.matmul(
                        rb2[:, c0:c0 + 256], sel_sb, rb16[:, i * 256:(i + 1) * 256],
                        start=True, stop=True, skip_group_check=True,
                    )
                    nc.vector.tensor_mul(
                        att[st][j][:, :], att[st][j][:, :], rb2[:, c0:c0 + 256]
                    )

            def new_scat(st, b, npairs):
                return sb.tile([1, npairs * 512], f32, tag=f"scat{st}_{b}",
                               name=f"scat{st}_{b}")

            # ---- S1: q cb0 ----
            for m in range(4):
                for th in q_group(0, m):
                    th()
                # gate late weight DMAs to S1 progress
                if m == 0:
                    gated_dma(wq4, 4 * 4096, qT[0][0:1, 0:8])
                elif m == 1:
                    gated_dma(wq5, 5 * 4096, qT[1][0:1, 0:8])
                elif m == 2:
                    gated_dma(wq1, 1 * 4096, qT[2][0:1, 0:8])
                elif m == 3:
                    gated_dma(wq3, 3 * 4096, qT[3][0:1, 0:8])
            # ---- S2: k cb2 ----
            for gi, (m, n) in enumerate([(m, n) for m in range(4) for n in range(2)]):
                for th in k_group(2, m, n):
                    th()
                if gi == 1:
                    gated_dma(woA, 6 * 4096, kT[0][0:1, 0:8])
                elif gi == 3:
                    gated_dma(woB, 7 * 4096, kT[0][0:1, 8:16])
            # ones columns for v tiles (vector, after cst)
            for t in range(5):
                v_ones = vt[t].rearrange("p (h c) -> p h c", c=65)[:, :, 64]
                nc.vector.tensor_copy(v_ones, ones_sb[:])
            # ---- S3: v half0 t0-2 ----
            for t in range(3):
                for th in v_group(t, 0):
                    th()

            # ---- S4/S5: st0 attention interleaved with remaining proj ----
            # order matters: st0 pair j consumes kT[4+m]/qT[4+m] at slot 8+2m
            # and v-half1 of vt0-2 from pair 4 on; writers must be EMITTED
            # before their readers (tile deps snapshot at emission).
            fills = []
            for m in range(4):
                for n in range(2):
                    fills.extend(k_group(3, m, n))
                if m < 3:
                    fills.extend(v_group(m, 1))
                fills.extend(q_group(1, m))
            for t in range(3, 5):
                fills.extend(v_group(t, 0))
                fills.extend(v_group(t, 1))

            fi = [0]

            def drain(k, fills=fills, fi=fi):
                while k > 0 and fi[0] < len(fills):
                    fills[fi[0]]()
                    fi[0] += 1
                    k -= 1

            scat0_b0 = new_scat(0, 0, 4)
            scat0_b1 = new_scat(0, 1, 4)
            rb16_00 = [None]
            for j in range(8):
                emit_qk(0, 2 * j)
                drain(10)
                emit_qk(0, 2 * j + 1)
                drain(10)
                if j >= 1:
                    jj = j - 1
                    emit_av_pair(0, jj, scat0_b0 if jj < 4 else scat0_b1, jj % 4, "s")
                    if jj == 3:
                        rb16_00[0] = den_start(0, [0, 1, 2, 3], scat0_b0)
                    elif jj == 4:
                        den_finish(0, [0, 1, 2, 3], rb16_00[0])
            drain(10 ** 9)
            emit_av_pair(0, 7, scat0_b1, 3, "s")
            rb16_01 = den_start(0, [4, 5, 6, 7], scat0_b1)

            # ---- S6: st1 attention interleaved with po2(st0) ----
            fills2 = []
            for g in range(4):
                fills2.extend(po2_group(0, g))
            fi2 = [0]

            def drain2(k, fi2=fi2):
                while k > 0 and fi2[0] < len(fills2):
                    fills2[fi2[0]]()
                    fi2[0] += 1
                    k -= 1

            scat1_b = [new_scat(1, 0, 4), new_scat(1, 1, 2),
                       new_scat(1, 2, 1), new_scat(1, 3, 1)]

            def st1_scat(j):
                if j < 4:
                    return scat1_b[0], j
                if j < 6:
                    return scat1_b[1], j - 4
                return scat1_b[j - 4], 0

            rb16_1 = {}
            for j in range(8):
                emit_qk(1, 2 * j)
                if j >= 2:
                    drain2(5)
                emit_qk(1, 2 * j + 1)
                if j >= 2:
                    drain2(5)
                if j == 1:
                    den_finish(0, [4, 5, 6, 7], rb16_01)
                if j >= 1:
                    jj = j - 1
                    sc_b, pib = st1_scat(jj)
                    emit_av_pair(1, jj, sc_b, pib, "v")
                    if jj == 3:
                        rb16_1[0] = den_start(1, [0, 1, 2, 3], scat1_b[0])
                    elif jj == 4:
                        den_finish(1, [0, 1, 2, 3], rb16_1[0])
                    elif jj == 5:
                        rb16_1[1] = den_start(1, [4, 5], scat1_b[1])
                    elif jj == 6:
                        den_finish(1, [4, 5], rb16_1[1])
                        rb16_1[2] = den_start(1, [6], scat1_b[2])
            drain2(10 ** 9)
            sc_b, pib = st1_scat(7)
            emit_av_pair(1, 7, sc_b, pib, "v")
            den_finish(1, [6], rb16_1[2])
            rb16_1[3] = den_start(1, [7], scat1_b[3])
            den_finish(1, [7], rb16_1[3])
            nc.sync.dma_start(out=out_d[:, 0:2048], in_=ot[0][:])

            # ---- S7: po2(st1) + output DMAs ----
            for g in range(4):
                for th in po2_group(1, g):
                    th()
                if g == 1:
                    nc.sync.dma_start(out=out_d[:, 2048:3072], in_=ot[1][:, 0:1024])
                elif g == 2:
                    nc.sync.dma_start(out=out_d[:, 3072:3584], in_=ot[1][:, 1024:1536])
            nc.sync.dma_start(out=out_d[:, 3584:4096], in_=ot[1][:, 1536:2048])

    nc.finalize()
    return nc


def _get_nc():
    if "nc" not in _cached:
        _cached["nc"] = _build()
    return _cached["nc"]


def _core_inputs(x, w_qkv, w_out):
    # shared, host-prebaked weight layouts (bf16, exact SBUF layout).
    # cb0 is m-major (4 contiguous 1024-col chunks); cb1..5 are k-major.
    Wq = w_qkv.reshape(8, 128, 6, 512)
    blocks = [Wq[:, :, 0, :].reshape(8, 128, 4, 128).transpose(1, 2, 0, 3).reshape(128, 4096)]
    for cb in range(1, 6):
        blocks.append(Wq[:, :, cb, :].transpose(1, 0, 2).reshape(128, 4096))
    wq_h = np.ascontiguousarray(np.concatenate(blocks, axis=1)).astype(BF16)
    wo_h = np.ascontiguousarray(
        w_out.reshape(8, 128, 1024).transpose(1, 0, 2).reshape(128, 8192)
    ).astype(BF16)

    in_maps = []
    for c in range(NCORES):
        b, qs = c // CPB, (c % CPB) * SLOC
        xs = np.zeros((TLOC, D), dtype=np.float32)
        lo = max(0, qs - HALO)
        xs[HALO - (qs - lo):] = x[b, lo:qs + SLOC]
        xt_h = np.ascontiguousarray(
            xs.T.reshape(8, 128, TLOC).transpose(1, 0, 2).reshape(128, 8 * TLOC)
        ).astype(BF16)

        # binary {0,1} masks multiplying exp'd scores.
        i = np.arange(256)[None, None, None, :]
        j = np.arange(128)[None, None, :, None]
        st = np.arange(NST)[:, None, None, None]
        r = np.arange(3)[None, :, None, None]
        qg = qs + st * 256 + i
        kg = qs + st * 256 - HALO + r * 128 + j
        allowed = (kg <= qg) & (kg > qg - WINDOW) & (kg >= 0)
        m3 = allowed.astype(np.float32)
        mask = np.empty((NST, 128, 512), dtype=np.float32)
        mask[:, :, 0:128] = m3[:, 0, :, 0:128]
        mask[:, :, 128:256] = m3[:, 2, :, 128:256]
        mask[:, :, 256:512] = m3[:, 1]
        cst_h = np.zeros((128, NST * 512 + 16 + 128), dtype=BF16)
        cst_h[:, 0:512] = mask[0]
        cst_h[:, 512:1024] = mask[1]
        cst_h[:, 1024:1040] = 1.0
        cst_h[0, 1040:1104] = 1.0    # sel row0: partitions 0..63
        cst_h[1, 1104:1168] = 1.0    # sel row1: partitions 64..127

        in_maps.append({"xt": xt_h, "wq": wq_h, "wo": wo_h, "cst": cst_h})
    return in_maps


def kernel(x, w_qkv, w_out, _trace=False, _trace_kwargs=None):
    from concourse.bass_utils import run_bass_kernel_spmd

    x = np.asarray(x, dtype=np.float32)
    w_qkv = np.asarray(w_qkv, dtype=np.float32)
    w_out = np.asarray(w_out, dtype=np.float32)
    nc = _get_nc()
    in_maps = _core_inputs(x, w_qkv, w_out)
    res = run_bass_kernel_spmd(
        nc, in_maps, list(range(NCORES)), trace=_trace, **(_trace_kwargs or {})
    )
    out = np.empty((B, S, D), dtype=np.float32)
    for c in range(NCORES):
        b, qs = c // CPB, (c % CPB) * SLOC
        o = np.asarray(res.results[c]["out"], dtype=np.float32)
        out[b, qs:qs + SLOC] = (
            o.reshape(128, NST, 8, 256).transpose(1, 3, 2, 0).reshape(SLOC, D)
        )
    if _trace:
        return out, res
    return out


# revision 20
# speedup vs baseline: 1.0063x; 1.0063x over previous
"""Local causal (sliding-window) attention on 8 Trainium2 NeuronCores.

Sequence-parallel: each core owns 512 consecutive query tokens of one batch
element (cores 0-3 -> batch 0, 4-7 -> batch 1) plus a 128-token halo whose
k/v are recomputed locally, so no inter-core communication is needed.

All matmuls run in bf16 (full PE rate at any moving size, half the HBM
bytes of fp32); PSUM accumulates fp32. Every input is host-prebaked into
the exact SBUF layout so each tensor arrives in a few fully contiguous
DMAs, issued in consumption order with dependency-gated staggering so the
round-robin DMA engine doesn't dilute early transfers with late weights.

The emission schedule software-pipelines engines: st0's attention pairs are
interleaved with the remaining projection groups (q-cb1, k-cb3, v halves),
st0's output projection fills st1's attention gaps, and denominators are
processed in sub-batches (batched SBUF->SBUF DMA to a [2n,256] tile,
reciprocal_approx_fast, casting DMA back, GpSimd partition-broadcasts) so
their serial chain hides under PE work.
"""

import sys

sys.path.insert(0, "/opt/trn_rl_repo")
import numpy as np
import ml_dtypes

BF16 = ml_dtypes.bfloat16

B, S, D = 2, 2048, 1024
H, DH = 16, 64
WINDOW = 128
NCORES = 8
SLOC = 512
HALO = 128
TLOC = SLOC + HALO
NST = 2
CPB = NCORES // B

_cached = {}


def _build():
    import concourse.bacc as bacc
    import concourse.mybir as mybir
    import concourse.tile as tile

    f32 = mybir.dt.float32
    bf16 = mybir.dt.bfloat16
    AF = mybir.ActivationFunctionType

    nc = bacc.Bacc(None)
    CSTW = NST * 512 + 16 + 128   # masks | ones16 | sel[2,128]
    xt_d = nc.declare_dram_parameter("xt", [128, 8 * TLOC], bf16, isOutput=False)
    wq_d = nc.declare_dram_parameter("wq", [128, 6 * 4096], bf16, isOutput=False)
    wo_d = nc.declare_dram_parameter("wo", [128, 8192], bf16, isOutput=False)
    cst_d = nc.declare_dram_parameter("cst", [128, CSTW], bf16, isOutput=False)
    out_d = nc.declare_dram_parameter("out", [128, NST * 2048], bf16, isOutput=True)

    with tile.TileContext(nc) as tc:
        with (
            tc.tile_pool(name="sb", bufs=1) as sb,
            tc.tile_pool(name="pjps", bufs=1, space="PSUM") as pjps,
            tc.tile_pool(name="scps", bufs=1, space="PSUM") as scps,
            tc.tile_pool(name="avps", bufs=1, space="PSUM") as avps,
        ):
            # ---- head DMAs, consumption-ordered. wq0 is m-major (4 chunks
            # so the first q-group gates on only 0.25MB of weights); xt in 2
            # halves so the first k-accumulation starts after half the x.
            wq0m = [sb.tile([128, 1024], bf16, tag=f"wq0m{m}", name=f"wq0m{m}")
                    for m in range(4)]
            nc.sync.dma_start(out=wq0m[0][:], in_=wq_d[:, 0:1024])
            xta = sb.tile([128, 4 * TLOC], bf16, tag="xta", name="xta")
            nc.sync.dma_start(out=xta[:], in_=xt_d[:, 0:4 * TLOC])
            xtb = sb.tile([128, 4 * TLOC], bf16, tag="xtb", name="xtb")
            nc.sync.dma_start(out=xtb[:], in_=xt_d[:, 4 * TLOC:8 * TLOC])

            def xt_sl(k, c0, c1):
                t = xta if k < 4 else xtb
                kk = k % 4
                return t[:, kk * TLOC + c0:kk * TLOC + c1]

            for m in range(1, 4):
                nc.sync.dma_start(
                    out=wq0m[m][:], in_=wq_d[:, m * 1024:(m + 1) * 1024]
                )
            cst = sb.tile([128, CSTW], bf16, tag="cst", name="cst")
            nc.sync.dma_start(out=cst[:], in_=cst_d[:])
            wq2 = sb.tile([128, 4096], bf16, tag="wq2", name="wq2")
            nc.sync.dma_start(out=wq2[:], in_=wq_d[:, 2 * 4096:3 * 4096])
            # later weight tiles are declared now but DMA'd behind a tiny
            # WAW "gate" write that keys each transfer to pipeline progress,
            # so early transfers keep full DMA bandwidth.
            wq4 = sb.tile([128, 4096], bf16, tag="wq4", name="wq4")
            wq5 = sb.tile([128, 4096], bf16, tag="wq5", name="wq5")
            wq1 = sb.tile([128, 4096], bf16, tag="wq1", name="wq1")
            wq3 = sb.tile([128, 4096], bf16, tag="wq3", name="wq3")
            woA = sb.tile([128, 4096], bf16, tag="woA", name="woA")
            woB = sb.tile([128, 4096], bf16, tag="woB", name="woB")

            def gated_dma(dst, col0, key_ap):
                nc.vector.tensor_copy(dst[0:1, 0:8], key_ap)
                nc.sync.dma_start(out=dst[:], in_=wq_d[:, col0:col0 + 4096]
                                  if col0 < 6 * 4096 else wo_d[:, col0 - 6 * 4096:col0 - 6 * 4096 + 4096])

            msk = [cst[:, 0:512], cst[:, 512:1024]]
            ones_sb = cst[:, 1024:1040]
            sel_sb = cst[0:2, 1040:1168]   # [2,128]: row0 = p<64, row1 = p>=64

            qT = [sb.tile([128, SLOC], bf16, tag=f"qT{i}", name=f"qT{i}") for i in range(8)]
            kT = [sb.tile([128, TLOC], bf16, tag=f"kT{i}", name=f"kT{i}") for i in range(8)]
            vt = [sb.tile([128, 65 * H], bf16, tag=f"v{t}", name=f"v{t}") for t in range(5)]
            att = [[sb.tile([128, 256], bf16, tag=f"at{st}_{t}", name=f"at{st}_{t}")
                    for t in range(8)] for st in range(NST)]
            ot = [sb.tile([128, 2048], bf16, tag=f"ot{st}", name=f"ot{st}") for st in range(NST)]

            # ---- projection group emitters (thunk lists of single ops) ----
            def q_group(cb, m):
                ps = pjps.tile([128, 512], f32, tag="qk", bufs=2, name=f"psq{cb}_{m}")
                th = []
                for k in range(8):
                    def mm(k=k, ps=ps, cb=cb, m=m):
                        if cb == 0:
                            lhs = wq0m[m][:, k * 128:(k + 1) * 128]
                        else:
                            lhs = wq1[:, k * 512 + m * 128:k * 512 + (m + 1) * 128]
                        nc.tensor.matmul(
                            ps[:], lhs, xt_sl(k, HALO, TLOC),
                            start=(k == 0), stop=(k == 7),
                        )
                    th.append(mm)
                if cb == 0:
                    th.append(lambda ps=ps, m=m: nc.scalar.copy(qT[m][:], ps[:]))
                else:
                    th.append(lambda ps=ps, m=m: nc.vector.tensor_copy(qT[4 + m][:], ps[:]))
                return th

            def k_group(cb, m, n):
                w = wq2 if cb == 2 else wq3
                ps = pjps.tile([128, 320], f32, tag="qk", bufs=2, name=f"psk{cb}_{m}_{n}")
                th = []
                for k in range(8):
                    def mm(k=k, ps=ps, w=w, m=m, n=n):
                        nc.tensor.matmul(
                            ps[:], w[:, k * 512 + m * 128:k * 512 + (m + 1) * 128],
                            xt_sl(k, n * 320, (n + 1) * 320),
                            start=(k == 0), stop=(k == 7),
                        )
                    th.append(mm)
                def kcp(ps=ps, cb=cb, m=m, n=n):
                    dst = kT[(cb - 2) * 4 + m][:, n * 320:(n + 1) * 320]
                    if cb == 2:
                        nc.scalar.copy(dst, ps[:])
                    else:
                        nc.vector.tensor_copy(dst, ps[:])
                th.append(kcp)
                return th

            def v_group(t, half):
                w = wq4 if half == 0 else wq5
                ps = pjps.tile([128, 512], f32, tag="qk", bufs=2, name=f"psv{t}_{half}")
                th = []
                for k in range(8):
                    def mm(k=k, ps=ps, w=w, t=t):
                        nc.tensor.matmul(
                            ps[:], xt_sl(k, t * 128, (t + 1) * 128),
                            w[:, k * 512:(k + 1) * 512],
                            start=(k == 0), stop=(k == 7),
                        )
                    th.append(mm)

                def cp(ps=ps, t=t, half=half):
                    h0 = half * 8
                    dst = vt[t].rearrange("p (h c) -> p h c", c=65)[:, h0:h0 + 8, 0:64]
                    src2 = ps[:].rearrange("p (h c) -> p h c", c=64)
                    if half == 0 and t < 3:
                        nc.scalar.copy(dst, src2)
                    else:
                        nc.vector.tensor_copy(dst, src2)
                th.append(cp)
                return th

            def po2_group(st, g):
                q0 = st * 256
                po = pjps.tile([128, 512], f32, tag="qk", bufs=2, name=f"po{st}_{g}")
                th = []
                for half in range(2):
                    m = 2 * g + half
                    c0 = half * 256
                    for k in range(8):
                        def mm(k=k, po=po, m=m, c0=c0, st=st, q0=q0):
                            wo = woA if k < 4 else woB
                            kk = k % 4
                            nc.tensor.matmul(
                                po[:, c0:c0 + 256],
                                wo[:, kk * 1024 + m * 128:kk * 1024 + (m + 1) * 128],
                                att[st][k][:, :],
                                start=(k == 0), stop=(k == 7),
                                skip_group_check=True,
                            )
                        th.append(mm)
                def pcp(po=po, st=st, g=g):
                    dst = ot[st][:, g * 512:(g + 1) * 512]
                    if st == 0:
                        nc.vector.tensor_copy(dst, po[:])
                    else:
                        nc.scalar.copy(dst, po[:])
                th.append(pcp)
                return th

            # ---- attention emitters ----
            pend = {}

            def emit_qk(st, h):
                q0 = st * 256
                jb = st * 2
                t, poff = h // 2, (h % 2) * 64
                sc = scps.tile([128, 512], f32, tag="sc", bufs=3, name=f"sc{st}_{h}")
                nc.tensor.matmul(
                    sc[:, 256:512],
                    kT[t][poff:poff + 64, (jb + 1) * 128:(jb + 2) * 128],
                    qT[t][poff:poff + 64, q0:q0 + 256],
                    start=True, stop=False, skip_group_check=True,
                )
                nc.tensor.matmul(
                    sc[:, 0:128],
                    kT[t][poff:poff + 64, jb * 128:(jb + 1) * 128],
                    qT[t][poff:poff + 64, q0:q0 + 128],
                    start=True, stop=False, skip_group_check=True,
                )
                nc.tensor.matmul(
                    sc[:, 128:256],
                    kT[t][poff:poff + 64, (jb + 2) * 128:(jb + 3) * 128],
                    qT[t][poff:poff + 64, q0 + 128:q0 + 256],
                    start=True, stop=True, skip_group_check=True,
                )
                p = sb.tile([128, 512], bf16, tag="pp", bufs=8, name=f"p{st}_{h}")
                nc.scalar.activation(p[:], sc[:], AF.Exp, scale=0.125)
                eng = nc.gpsimd if (st == 1 and h % 2 == 1) else nc.vector
                eng.tensor_mul(p[:], p[:], msk[st])
                pend[(st, h)] = p

            def emit_av_pair(st, j, scat_b, pair_in_b, cast_eng):
                jb = st * 2
                t = j
                p0, p1 = pend.pop((st, 2 * j)), pend.pop((st, 2 * j + 1))
                av = avps.tile([65, 512], f32, tag="av", bufs=2, name=f"av{st}_{j}")
                for half, p in ((0, p0), (1, p1)):
                    c0 = half * 256
                    h = 2 * j + half
                    nc.tensor.matmul(
                        av[:, c0:c0 + 256], vt[jb + 1][:, h * 65:h * 65 + 65],
                        p[:, 256:512],
                        start=True, stop=False, skip_group_check=True,
                    )
                    nc.tensor.matmul(
                        av[:, c0:c0 + 128], vt[jb][:, h * 65:h * 65 + 65],
                        p[:, 0:128],
                        start=False, stop=False, skip_group_check=True,
                    )
                    nc.tensor.matmul(
                        av[:, c0 + 128:c0 + 256], vt[jb + 2][:, h * 65:h * 65 + 65],
                        p[:, 128:256],
                        start=False, stop=True, skip_group_check=True,
                    )
                dstv = scat_b[0:1, :].rearrange("p (h r) -> p h r", h=2)[
                    :, :, pair_in_b * 256:(pair_in_b + 1) * 256]
                nc.scalar.copy(dstv, av[64:65, :].rearrange("p (h c) -> p h c", h=2))
                if cast_eng == "s":
                    nc.scalar.copy(att[st][t][0:64, :], av[0:64, 0:256])
                    nc.scalar.copy(att[st][t][64:128, :], av[0:64, 256:512])
                else:
                    nc.vector.tensor_copy(att[st][t][0:64, :], av[0:64, 0:256])
                    nc.vector.tensor_copy(att[st][t][64:128, :], av[0:64, 256:512])

            def den_start(st, pairs, scat_b):
                n = len(pairs)
                s_b = sb.tile([2, n * 256], f32, tag="s_b", bufs=3, name=f"s{st}_{pairs[0]}")
                nc.sync.dma_start(out=s_b[:], in_=scat_b[0:1, :])
                r_b = sb.tile([2, n * 256], f32, tag="r_b", bufs=3, name=f"r{st}_{pairs[0]}")
                nc.vector.reciprocal_approx_fast(out=r_b[:], in_=s_b[:])
                rb16 = sb.tile([2, n * 256], bf16, tag="rb16", bufs=3, name=f"rb16_{st}_{pairs[0]}")
                nc.vector.tensor_copy(rb16[:], r_b[:])
                return rb16

            def den_finish(st, pairs, rb16):
                rb2 = None
                for i, j in enumerate(pairs):
                    if i % 2 == 0:
                        rb2 = scps.tile([128, 512], f32, tag="rb2", bufs=1,
                                        name=f"rb2_{st}_{j}")
                    c0 = (i % 2) * 256
                    nc.tensor.matmul(
                        rb2[:, c0:c0 + 256], sel_sb, rb16[:, i * 256:(i + 1) * 256],
                        start=True, stop=True, skip_group_check=True,
                    )
                    nc.vector.tensor_mul(
                        att[st][j][:, :], att[st][j][:, :], rb2[:, c0:c0 + 256]
                    )

            def new_scat(st, b, npairs):
                return sb.tile([1, npairs * 512], f32, tag=f"scat{st}_{b}",
                               name=f"scat{st}_{b}")

            # ---- S1: q cb0 ----
            for m in range(4):
                for th in q_group(0, m):
                    th()
                # gate late weight DMAs to S1 progress
                if m == 0:
                    gated_dma(wq4, 4 * 4096, qT[0][0:1, 0:8])
                elif m == 1:
                    gated_dma(wq5, 5 * 4096, qT[1][0:1, 0:8])
                elif m == 2:
                    gated_dma(wq1, 1 * 4096, qT[2][0:1, 0:8])
                elif m == 3:
                    gated_dma(wq3, 3 * 4096, qT[3][0:1, 0:8])
            # ---- S2: k cb2 ----
            for gi, (m, n) in enumerate([(m, n) for m in range(4) for n in range(2)]):
                for th in k_group(2, m, n):
                    th()
                if gi == 1:
                    gated_dma(woA, 6 * 4096, kT[0][0:1, 0:8])
                elif gi == 3:
                    gated_dma(woB, 7 * 4096, kT[0][0:1, 8:16])
            # ones columns for v tiles (vector, after cst)
            for t in range(5):
                v_ones = vt[t].rearrange("p (h c) -> p h c", c=65)[:, :, 64]
                nc.vector.tensor_copy(v_ones, ones_sb[:])
            # ---- S3: v half0 t0-2 ----
            for t in range(3):
                for th in v_group(t, 0):
                    th()

            # ---- S4/S5: st0 attention interleaved with remaining proj ----
            # order matters: st0 pair j consumes kT[4+m]/qT[4+m] at slot 8+2m
            # and v-half1 of vt0-2 from pair 4 on; writers must be EMITTED
            # before their readers (tile deps snapshot at emission).
            fills = []
            for m in range(4):
                for n in range(2):
                    fills.extend(k_group(3, m, n))
                if m < 3:
                    fills.extend(v_group(m, 1))
                fills.extend(q_group(1, m))
            for t in range(3, 5):
                fills.extend(v_group(t, 0))
                fills.extend(v_group(t, 1))

            fi = [0]

            def drain(k, fills=fills, fi=fi):
                while k > 0 and fi[0] < len(fills):
                    fills[fi[0]]()
                    fi[0] += 1
                    k -= 1

            scat0_b0 = new_scat(0, 0, 4)
            scat0_b1 = new_scat(0, 1, 4)
            rb16_00 = [None]
            for j in range(8):
                emit_qk(0, 2 * j)
                drain(10)
                emit_qk(0, 2 * j + 1)
                drain(10)
                if j >= 1:
                    jj = j - 1
                    emit_av_pair(0, jj, scat0_b0 if jj < 4 else scat0_b1, jj % 4, "s")
                    if jj == 3:
                        rb16_00[0] = den_start(0, [0, 1, 2, 3], scat0_b0)
                    elif jj == 4:
                        den_finish(0, [0, 1, 2, 3], rb16_00[0])
            drain(10 ** 9)
            emit_av_pair(0, 7, scat0_b1, 3, "s")
            rb16_01 = den_start(0, [4, 5, 6, 7], scat0_b1)

            # ---- S6: st1 attention interleaved with po2(st0) ----
            fills2 = []
            for g in range(4):
                fills2.extend(po2_group(0, g))
            fi2 = [0]

            def drain2(k, fi2=fi2):
                while k > 0 and fi2[0] < len(fills2):
                    fills2[fi2[0]]()
                    fi2[0] += 1
                    k -= 1

            scat1_b = [new_scat(1, 0, 4), new_scat(1, 1, 2),
                       new_scat(1, 2, 1), new_scat(1, 3, 1)]

            def st1_scat(j):
                if j < 4:
                    return scat1_b[0], j
                if j < 6:
                    return scat1_b[1], j - 4
                return scat1_b[j - 4], 0

            rb16_1 = {}
            for j in range(8):
                emit_qk(1, 2 * j)
                if j >= 2:
                    drain2(5)
                emit_qk(1, 2 * j + 1)
                if j >= 2:
                    drain2(5)
                if j == 1:
                    den_finish(0, [4, 5, 6, 7], rb16_01)
                if j >= 1:
                    jj = j - 1
                    sc_b, pib = st1_scat(jj)
                    emit_av_pair(1, jj, sc_b, pib, "v")
                    if jj == 3:
                        rb16_1[0] = den_start(1, [0, 1, 2, 3], scat1_b[0])
                    elif jj == 4:
                        den_finish(1, [0, 1, 2, 3], rb16_1[0])
                    elif jj == 5:
                        rb16_1[1] = den_start(1, [4, 5], scat1_b[1])
                    elif jj == 6:
                        den_finish(1, [4, 5], rb16_1[1])
                        rb16_1[2] = den_start(1, [6], scat1_b[2])
            drain2(10 ** 9)
            sc_b, pib = st1_scat(7)
            emit_av_pair(1, 7, sc_b, pib, "v")
            den_finish(1, [6], rb16_1[2])
            rb16_1[3] = den_start(1, [7], scat1_b[3])
            den_finish(1, [7], rb16_1[3])
            nc.sync.dma_start(out=out_d[:, 0:2048], in_=ot[0][:])

            # ---- S7: po2(st1) + output DMAs ----
            for g in range(4):
                for th in po2_group(1, g):
                    th()
                if g == 1:
                    nc.sync.dma_start(out=out_d[:, 2048:3072], in_=ot[1][:, 0:1024])
                elif g == 2:
                    nc.sync.dma_start(out=out_d[:, 3072:3584], in_=ot[1][:, 1024:1536])
            nc.sync.dma_start(out=out_d[:, 3584:4096], in_=ot[1][:, 1536:2048])

    nc.finalize()
    return nc


def _get_nc():
    if "nc" not in _cached:
        _cached["nc"] = _build()
    return _cached["nc"]


def _core_inputs(x, w_qkv, w_out):
    # shared, host-prebaked weight layouts (bf16, exact SBUF layout).
    # cb0 is m-major (4 contiguous 1024-col chunks); cb1..5 are k-major.
    Wq = w_qkv.reshape(8, 128, 6, 512)
    blocks = [Wq[:, :, 0, :].reshape(8, 128, 4, 128).transpose(1, 2, 0, 3).reshape(128, 4096)]
    for cb in range(1, 6):
        blocks.append(Wq[:, :, cb, :].transpose(1, 0, 2).reshape(128, 4096))
    wq_h = np.ascontiguousarray(np.concatenate(blocks, axis=1)).astype(BF16)
    wo_h = np.ascontiguousarray(
        w_out.reshape(8, 128, 1024).transpose(1, 0, 2).reshape(128, 8192)
    ).astype(BF16)

    in_maps = []
    for c in range(NCORES):
        b, qs = c // CPB, (c % CPB) * SLOC
        xs = np.zeros((TLOC, D), dtype=np.float32)
        lo = max(0, qs - HALO)
        xs[HALO - (qs - lo):] = x[b, lo:qs + SLOC]
        xt_h = np.ascontiguousarray(
            xs.T.reshape(8, 128, TLOC).transpose(1, 0, 2).reshape(128, 8 * TLOC)
        ).astype(BF16)

        # binary {0,1} masks multiplying exp'd scores.
        i = np.arange(256)[None, None, None, :]
        j = np.arange(128)[None, None, :, None]
        st = np.arange(NST)[:, None, None, None]
        r = np.arange(3)[None, :, None, None]
        qg = qs + st * 256 + i
        kg = qs + st * 256 - HALO + r * 128 + j
        allowed = (kg <= qg) & (kg > qg - WINDOW) & (kg >= 0)
        m3 = allowed.astype(np.float32)
        mask = np.empty((NST, 128, 512), dtype=np.float32)
        mask[:, :, 0:128] = m3[:, 0, :, 0:128]
        mask[:, :, 128:256] = m3[:, 2, :, 128:256]
        mask[:, :, 256:512] = m3[:, 1]
        cst_h = np.zeros((128, NST * 512 + 16 + 128), dtype=BF16)
        cst_h[:, 0:512] = mask[0]
        cst_h[:, 512:1024] = mask[1]
        cst_h[:, 1024:1040] = 1.0
        cst_h[0, 1040:1104] = 1.0    # sel row0: partitions 0..63
        cst_h[1, 1104:1168] = 1.0    # sel row1: partitions 64..127

        in_maps.append({"xt": xt_h, "wq": wq_h, "wo": wo_h, "cst": cst_h})
    return in_maps


def kernel(x, w_qkv, w_out, _trace=False, _trace_kwargs=None):
    from concourse.bass_utils import run_bass_kernel_spmd

    x = np.asarray(x, dtype=np.float32)
    w_qkv = np.asarray(w_qkv, dtype=np.float32)
    w_out = np.asarray(w_out, dtype=np.float32)
    nc = _get_nc()
    in_maps = _core_inputs(x, w_qkv, w_out)
    res = run_bass_kernel_spmd(
        nc, in_maps, list(range(NCORES)), trace=_trace, **(_trace_kwargs or {})
    )
    out = np.empty((B, S, D), dtype=np.float32)
    for c in range(NCORES):
        b, qs = c // CPB, (c % CPB) * SLOC
        o = np.asarray(res.results[c]["out"], dtype=np.float32)
        out[b, qs:qs + SLOC] = (
            o.reshape(128, NST, 8, 256).transpose(1, 3, 2, 0).reshape(SLOC, D)
        )
    if _trace:
        return out, res
    return out


# revision 21
# speedup vs baseline: 1.0263x; 1.0199x over previous
"""Local causal (sliding-window) attention on 8 Trainium2 NeuronCores.

Sequence-parallel: each core owns 512 consecutive query tokens of one batch
element (cores 0-3 -> batch 0, 4-7 -> batch 1) plus a 128-token halo whose
k/v are recomputed locally, so no inter-core communication is needed.

All matmuls run in bf16 (full PE rate at any moving size, half the HBM
bytes of fp32); PSUM accumulates fp32. Every input is host-prebaked into
the exact SBUF layout so each tensor arrives in a few fully contiguous
DMAs, issued in consumption order with dependency-gated staggering so the
round-robin DMA engine doesn't dilute early transfers with late weights.

The emission schedule software-pipelines engines: st0's attention pairs are
interleaved with the remaining projection groups (q-cb1, k-cb3, v halves),
st0's output projection fills st1's attention gaps, and denominators are
processed in sub-batches (batched SBUF->SBUF DMA to a [2n,256] tile,
reciprocal_approx_fast, casting DMA back, GpSimd partition-broadcasts) so
their serial chain hides under PE work.
"""

import sys

sys.path.insert(0, "/opt/trn_rl_repo")
import numpy as np
import ml_dtypes

BF16 = ml_dtypes.bfloat16

B, S, D = 2, 2048, 1024
H, DH = 16, 64
WINDOW = 128
NCORES = 8
SLOC = 512
HALO = 128
TLOC = SLOC + HALO
NST = 2
CPB = NCORES // B

_cached = {}


def _build():
    import concourse.bacc as bacc
    import concourse.mybir as mybir
    import concourse.tile as tile

    f32 = mybir.dt.float32
    bf16 = mybir.dt.bfloat16
    AF = mybir.ActivationFunctionType

    nc = bacc.Bacc(None)
    CSTW = NST * 512 + 16 + 128   # masks | ones16 | sel[2,128]
    xt_d = nc.declare_dram_parameter("xt", [128, 8 * TLOC], bf16, isOutput=False)
    wq_d = nc.declare_dram_parameter("wq", [128, 6 * 4096], bf16, isOutput=False)
    wo_d = nc.declare_dram_parameter("wo", [128, 8192], bf16, isOutput=False)
    cst_d = nc.declare_dram_parameter("cst", [128, CSTW], bf16, isOutput=False)
    out_d = nc.declare_dram_parameter("out", [128, NST * 2048], bf16, isOutput=True)

    with tile.TileContext(nc) as tc:
        with (
            tc.tile_pool(name="sb", bufs=1) as sb,
            tc.tile_pool(name="pjps", bufs=1, space="PSUM") as pjps,
            tc.tile_pool(name="scps", bufs=1, space="PSUM") as scps,
            tc.tile_pool(name="avps", bufs=1, space="PSUM") as avps,
        ):
            # ---- head DMAs, consumption-ordered. wq0 is m-major (4 chunks
            # so the first q-group gates on only 0.25MB of weights); xt in 2
            # halves so the first k-accumulation starts after half the x.
            wq0m = [sb.tile([128, 1024], bf16, tag=f"wq0m{m}", name=f"wq0m{m}")
                    for m in range(4)]
            nc.sync.dma_start(out=wq0m[0][:], in_=wq_d[:, 0:1024])
            xta = sb.tile([128, 4 * TLOC], bf16, tag="xta", name="xta")
            nc.sync.dma_start(out=xta[:], in_=xt_d[:, 0:4 * TLOC])
            xtb = sb.tile([128, 4 * TLOC], bf16, tag="xtb", name="xtb")
            nc.sync.dma_start(out=xtb[:], in_=xt_d[:, 4 * TLOC:8 * TLOC])

            def xt_sl(k, c0, c1):
                t = xta if k < 4 else xtb
                kk = k % 4
                return t[:, kk * TLOC + c0:kk * TLOC + c1]

            for m in range(1, 4):
                nc.sync.dma_start(
                    out=wq0m[m][:], in_=wq_d[:, m * 1024:(m + 1) * 1024]
                )
            cst = sb.tile([128, CSTW], bf16, tag="cst", name="cst")
            nc.sync.dma_start(out=cst[:], in_=cst_d[:])
            wq2 = sb.tile([128, 4096], bf16, tag="wq2", name="wq2")
            nc.sync.dma_start(out=wq2[:], in_=wq_d[:, 2 * 4096:3 * 4096])
            # later weight tiles are declared now but DMA'd behind a tiny
            # WAW "gate" write that keys each transfer to pipeline progress,
            # so early transfers keep full DMA bandwidth.
            wq4 = sb.tile([128, 4096], bf16, tag="wq4", name="wq4")
            wq5 = sb.tile([128, 4096], bf16, tag="wq5", name="wq5")
            wq1 = sb.tile([128, 4096], bf16, tag="wq1", name="wq1")
            wq3 = sb.tile([128, 4096], bf16, tag="wq3", name="wq3")
            woA = sb.tile([128, 4096], bf16, tag="woA", name="woA")
            woB = sb.tile([128, 4096], bf16, tag="woB", name="woB")

            def gated_dma(dst, col0, key_ap):
                nc.vector.tensor_copy(dst[0:1, 0:8], key_ap)
                nc.sync.dma_start(out=dst[:], in_=wq_d[:, col0:col0 + 4096]
                                  if col0 < 6 * 4096 else wo_d[:, col0 - 6 * 4096:col0 - 6 * 4096 + 4096])

            msk = [cst[:, 0:512], cst[:, 512:1024]]
            ones_sb = cst[:, 1024:1040]
            sel_sb = cst[0:2, 1040:1168]   # [2,128]: row0 = p<64, row1 = p>=64

            qT = [sb.tile([128, SLOC], bf16, tag=f"qT{i}", name=f"qT{i}") for i in range(8)]
            kT = [sb.tile([128, TLOC], bf16, tag=f"kT{i}", name=f"kT{i}") for i in range(8)]
            vt = [sb.tile([128, 65 * H], bf16, tag=f"v{t}", name=f"v{t}") for t in range(5)]
            att = [[sb.tile([128, 256], bf16, tag=f"at{st}_{t}", name=f"at{st}_{t}")
                    for t in range(8)] for st in range(NST)]
            ot = [sb.tile([128, 2048], bf16, tag=f"ot{st}", name=f"ot{st}") for st in range(NST)]

            # ---- projection group emitters (thunk lists of single ops) ----
            def q_group(cb, m):
                ps = pjps.tile([128, 512], f32, tag="qk", bufs=2, name=f"psq{cb}_{m}")
                th = []
                for k in range(8):
                    def mm(k=k, ps=ps, cb=cb, m=m):
                        if cb == 0:
                            lhs = wq0m[m][:, k * 128:(k + 1) * 128]
                        else:
                            lhs = wq1[:, k * 512 + m * 128:k * 512 + (m + 1) * 128]
                        nc.tensor.matmul(
                            ps[:], lhs, xt_sl(k, HALO, TLOC),
                            start=(k == 0), stop=(k == 7),
                        )
                    th.append(mm)
                if cb == 0:
                    th.append(lambda ps=ps, m=m: nc.scalar.copy(qT[m][:], ps[:]))
                else:
                    th.append(lambda ps=ps, m=m: nc.vector.tensor_copy(qT[4 + m][:], ps[:]))
                return th

            def k_group(cb, m, n):
                w = wq2 if cb == 2 else wq3
                ps = pjps.tile([128, 320], f32, tag="qk", bufs=2, name=f"psk{cb}_{m}_{n}")
                th = []
                for k in range(8):
                    def mm(k=k, ps=ps, w=w, m=m, n=n):
                        nc.tensor.matmul(
                            ps[:], w[:, k * 512 + m * 128:k * 512 + (m + 1) * 128],
                            xt_sl(k, n * 320, (n + 1) * 320),
                            start=(k == 0), stop=(k == 7),
                        )
                    th.append(mm)
                def kcp(ps=ps, cb=cb, m=m, n=n):
                    dst = kT[(cb - 2) * 4 + m][:, n * 320:(n + 1) * 320]
                    if cb == 2:
                        nc.scalar.copy(dst, ps[:])
                    else:
                        nc.vector.tensor_copy(dst, ps[:])
                th.append(kcp)
                return th

            def v_group(t, half):
                w = wq4 if half == 0 else wq5
                ps = pjps.tile([128, 512], f32, tag="qk", bufs=2, name=f"psv{t}_{half}")
                th = []
                for k in range(8):
                    def mm(k=k, ps=ps, w=w, t=t):
                        nc.tensor.matmul(
                            ps[:], xt_sl(k, t * 128, (t + 1) * 128),
                            w[:, k * 512:(k + 1) * 512],
                            start=(k == 0), stop=(k == 7),
                        )
                    th.append(mm)

                def cp(ps=ps, t=t, half=half):
                    h0 = half * 8
                    dst = vt[t].rearrange("p (h c) -> p h c", c=65)[:, h0:h0 + 8, 0:64]
                    src2 = ps[:].rearrange("p (h c) -> p h c", c=64)
                    if half == 0 and t < 3:
                        nc.scalar.copy(dst, src2)
                    else:
                        nc.vector.tensor_copy(dst, src2)
                th.append(cp)
                return th

            def po2_group(st, g):
                q0 = st * 256
                po = pjps.tile([128, 512], f32, tag="qk", bufs=2, name=f"po{st}_{g}")
                th = []
                for half in range(2):
                    m = 2 * g + half
                    c0 = half * 256
                    for k in range(8):
                        def mm(k=k, po=po, m=m, c0=c0, st=st, q0=q0):
                            wo = woA if k < 4 else woB
                            kk = k % 4
                            nc.tensor.matmul(
                                po[:, c0:c0 + 256],
                                wo[:, kk * 1024 + m * 128:kk * 1024 + (m + 1) * 128],
                                att[st][k][:, :],
                                start=(k == 0), stop=(k == 7),
                                skip_group_check=True,
                            )
                        th.append(mm)
                th.append(lambda po=po, st=st, g=g: nc.scalar.copy(
                    ot[st][:, g * 512:(g + 1) * 512], po[:]))
                return th

            # ---- attention emitters ----
            pend = {}

            def emit_qk(st, h):
                q0 = st * 256
                jb = st * 2
                t, poff = h // 2, (h % 2) * 64
                sc = scps.tile([128, 512], f32, tag="sc", bufs=3, name=f"sc{st}_{h}")
                nc.tensor.matmul(
                    sc[:, 256:512],
                    kT[t][poff:poff + 64, (jb + 1) * 128:(jb + 2) * 128],
                    qT[t][poff:poff + 64, q0:q0 + 256],
                    start=True, stop=False, skip_group_check=True,
                )
                nc.tensor.matmul(
                    sc[:, 0:128],
                    kT[t][poff:poff + 64, jb * 128:(jb + 1) * 128],
                    qT[t][poff:poff + 64, q0:q0 + 128],
                    start=True, stop=False, skip_group_check=True,
                )
                nc.tensor.matmul(
                    sc[:, 128:256],
                    kT[t][poff:poff + 64, (jb + 2) * 128:(jb + 3) * 128],
                    qT[t][poff:poff + 64, q0 + 128:q0 + 256],
                    start=True, stop=True, skip_group_check=True,
                )
                p = sb.tile([128, 512], bf16, tag="pp", bufs=8, name=f"p{st}_{h}")
                nc.scalar.activation(p[:], sc[:], AF.Exp, scale=0.125)
                eng = nc.gpsimd if (st == 1 and h % 2 == 1) else nc.vector
                eng.tensor_mul(p[:], p[:], msk[st])
                pend[(st, h)] = p

            def emit_av_pair(st, j, scat_b, pair_in_b, cast_eng):
                jb = st * 2
                t = j
                p0, p1 = pend.pop((st, 2 * j)), pend.pop((st, 2 * j + 1))
                av = avps.tile([65, 512], f32, tag="av", bufs=2, name=f"av{st}_{j}")
                for half, p in ((0, p0), (1, p1)):
                    c0 = half * 256
                    h = 2 * j + half
                    nc.tensor.matmul(
                        av[:, c0:c0 + 256], vt[jb + 1][:, h * 65:h * 65 + 65],
                        p[:, 256:512],
                        start=True, stop=False, skip_group_check=True,
                    )
                    nc.tensor.matmul(
                        av[:, c0:c0 + 128], vt[jb][:, h * 65:h * 65 + 65],
                        p[:, 0:128],
                        start=False, stop=False, skip_group_check=True,
                    )
                    nc.tensor.matmul(
                        av[:, c0 + 128:c0 + 256], vt[jb + 2][:, h * 65:h * 65 + 65],
                        p[:, 128:256],
                        start=False, stop=True, skip_group_check=True,
                    )
                dstv = scat_b[0:1, :].rearrange("p (h r) -> p h r", h=2)[
                    :, :, pair_in_b * 256:(pair_in_b + 1) * 256]
                nc.scalar.copy(dstv, av[64:65, :].rearrange("p (h c) -> p h c", h=2))
                if cast_eng == "s":
                    nc.scalar.copy(att[st][t][0:64, :], av[0:64, 0:256])
                    nc.scalar.copy(att[st][t][64:128, :], av[0:64, 256:512])
                else:
                    nc.vector.tensor_copy(att[st][t][0:64, :], av[0:64, 0:256])
                    nc.vector.tensor_copy(att[st][t][64:128, :], av[0:64, 256:512])

            def den_start(st, pairs, scat_b):
                n = len(pairs)
                s_b = sb.tile([2, n * 256], f32, tag="s_b", bufs=3, name=f"s{st}_{pairs[0]}")
                nc.sync.dma_start(out=s_b[:], in_=scat_b[0:1, :])
                r_b = sb.tile([2, n * 256], f32, tag="r_b", bufs=3, name=f"r{st}_{pairs[0]}")
                nc.vector.reciprocal_approx_fast(out=r_b[:], in_=s_b[:])
                rb16 = sb.tile([2, n * 256], bf16, tag="rb16", bufs=3, name=f"rb16_{st}_{pairs[0]}")
                nc.vector.tensor_copy(rb16[:], r_b[:])
                return rb16

            def den_finish(st, pairs, rb16):
                rb2 = None
                for i, j in enumerate(pairs):
                    if i % 2 == 0:
                        rb2 = scps.tile([128, 512], f32, tag="rb2", bufs=1,
                                        name=f"rb2_{st}_{j}")
                    c0 = (i % 2) * 256
                    nc.tensor.matmul(
                        rb2[:, c0:c0 + 256], sel_sb, rb16[:, i * 256:(i + 1) * 256],
                        start=True, stop=True, skip_group_check=True,
                    )
                    nc.vector.tensor_mul(
                        att[st][j][:, :], att[st][j][:, :], rb2[:, c0:c0 + 256]
                    )

            def new_scat(st, b, npairs):
                return sb.tile([1, npairs * 512], f32, tag=f"scat{st}_{b}",
                               name=f"scat{st}_{b}")

            # ---- S1: q cb0 ----
            for m in range(4):
                for th in q_group(0, m):
                    th()
                # gate late weight DMAs to S1 progress
                if m == 0:
                    gated_dma(wq4, 4 * 4096, qT[0][0:1, 0:8])
                elif m == 1:
                    gated_dma(wq5, 5 * 4096, qT[1][0:1, 0:8])
                elif m == 2:
                    gated_dma(wq1, 1 * 4096, qT[2][0:1, 0:8])
                elif m == 3:
                    gated_dma(wq3, 3 * 4096, qT[3][0:1, 0:8])
            # ---- S2: k cb2 ----
            for gi, (m, n) in enumerate([(m, n) for m in range(4) for n in range(2)]):
                for th in k_group(2, m, n):
                    th()
                if gi == 1:
                    gated_dma(woA, 6 * 4096, kT[0][0:1, 0:8])
                elif gi == 3:
                    gated_dma(woB, 7 * 4096, kT[0][0:1, 8:16])
            # ones columns for v tiles (vector, after cst)
            for t in range(5):
                v_ones = vt[t].rearrange("p (h c) -> p h c", c=65)[:, :, 64]
                nc.vector.tensor_copy(v_ones, ones_sb[:])
            # ---- S3: v half0 t0-2 ----
            for t in range(3):
                for th in v_group(t, 0):
                    th()

            # ---- S4/S5: st0 attention interleaved with remaining proj ----
            # order matters: st0 pair j consumes kT[4+m]/qT[4+m] at slot 8+2m
            # and v-half1 of vt0-2 from pair 4 on; writers must be EMITTED
            # before their readers (tile deps snapshot at emission).
            fills = []
            for m in range(4):
                for n in range(2):
                    fills.extend(k_group(3, m, n))
                if m < 3:
                    fills.extend(v_group(m, 1))
                fills.extend(q_group(1, m))
            for t in range(3, 5):
                fills.extend(v_group(t, 0))
                fills.extend(v_group(t, 1))

            fi = [0]

            def drain(k, fills=fills, fi=fi):
                while k > 0 and fi[0] < len(fills):
                    fills[fi[0]]()
                    fi[0] += 1
                    k -= 1

            scat0_b0 = new_scat(0, 0, 4)
            scat0_b1 = new_scat(0, 1, 4)
            rb16_00 = [None]
            for j in range(8):
                emit_qk(0, 2 * j)
                drain(10)
                emit_qk(0, 2 * j + 1)
                drain(10)
                if j >= 1:
                    jj = j - 1
                    emit_av_pair(0, jj, scat0_b0 if jj < 4 else scat0_b1, jj % 4, "s")
                    if jj == 3:
                        rb16_00[0] = den_start(0, [0, 1, 2, 3], scat0_b0)
                    elif jj == 4:
                        den_finish(0, [0, 1, 2, 3], rb16_00[0])
            drain(10 ** 9)
            emit_av_pair(0, 7, scat0_b1, 3, "s")
            rb16_01 = den_start(0, [4, 5, 6, 7], scat0_b1)

            # ---- S6: st1 attention interleaved with po2(st0) ----
            fills2 = []
            for g in range(4):
                fills2.extend(po2_group(0, g))
            fi2 = [0]

            def drain2(k, fi2=fi2):
                while k > 0 and fi2[0] < len(fills2):
                    fills2[fi2[0]]()
                    fi2[0] += 1
                    k -= 1

            scat1_b = [new_scat(1, 0, 4), new_scat(1, 1, 2),
                       new_scat(1, 2, 1), new_scat(1, 3, 1)]

            def st1_scat(j):
                if j < 4:
                    return scat1_b[0], j
                if j < 6:
                    return scat1_b[1], j - 4
                return scat1_b[j - 4], 0

            rb16_1 = {}
            for j in range(8):
                emit_qk(1, 2 * j)
                if j >= 2:
                    drain2(5)
                emit_qk(1, 2 * j + 1)
                if j >= 2:
                    drain2(5)
                if j == 1:
                    den_finish(0, [4, 5, 6, 7], rb16_01)
                if j >= 1:
                    jj = j - 1
                    sc_b, pib = st1_scat(jj)
                    emit_av_pair(1, jj, sc_b, pib, "v")
                    if jj == 3:
                        rb16_1[0] = den_start(1, [0, 1, 2, 3], scat1_b[0])
                    elif jj == 4:
                        den_finish(1, [0, 1, 2, 3], rb16_1[0])
                    elif jj == 5:
                        rb16_1[1] = den_start(1, [4, 5], scat1_b[1])
                    elif jj == 6:
                        den_finish(1, [4, 5], rb16_1[1])
                        rb16_1[2] = den_start(1, [6], scat1_b[2])
            drain2(10 ** 9)
            sc_b, pib = st1_scat(7)
            emit_av_pair(1, 7, sc_b, pib, "v")
            den_finish(1, [6], rb16_1[2])
            rb16_1[3] = den_start(1, [7], scat1_b[3])
            den_finish(1, [7], rb16_1[3])
            nc.sync.dma_start(out=out_d[:, 0:2048], in_=ot[0][:])

            # ---- S7: po2(st1) + output DMAs ----
            for g in range(4):
                for th in po2_group(1, g):
                    th()
                if g == 1:
                    nc.sync.dma_start(out=out_d[:, 2048:3072], in_=ot[1][:, 0:1024])
                elif g == 2:
                    nc.sync.dma_start(out=out_d[:, 3072:3584], in_=ot[1][:, 1024:1536])
            nc.sync.dma_start(out=out_d[:, 3584:4096], in_=ot[1][:, 1536:2048])

    nc.finalize()
    return nc


def _get_nc():
    if "nc" not in _cached:
        _cached["nc"] = _build()
    return _cached["nc"]


def _core_inputs(x, w_qkv, w_out):
    # shared, host-prebaked weight layouts (bf16, exact SBUF layout).
    # cb0 is m-major (4 contiguous 1024-col chunks); cb1..5 are k-major.
    Wq = w_qkv.reshape(8, 128, 6, 512)
    blocks = [Wq[:, :, 0, :].reshape(8, 128, 4, 128).transpose(1, 2, 0, 3).reshape(128, 4096)]
    for cb in range(1, 6):
        blocks.append(Wq[:, :, cb, :].transpose(1, 0, 2).reshape(128, 4096))
    wq_h = np.ascontiguousarray(np.concatenate(blocks, axis=1)).astype(BF16)
    wo_h = np.ascontiguousarray(
        w_out.reshape(8, 128, 1024).transpose(1, 0, 2).reshape(128, 8192)
    ).astype(BF16)

    in_maps = []
    for c in range(NCORES):
        b, qs = c // CPB, (c % CPB) * SLOC
        xs = np.zeros((TLOC, D), dtype=np.float32)
        lo = max(0, qs - HALO)
        xs[HALO - (qs - lo):] = x[b, lo:qs + SLOC]
        xt_h = np.ascontiguousarray(
            xs.T.reshape(8, 128, TLOC).transpose(1, 0, 2).reshape(128, 8 * TLOC)
        ).astype(BF16)

        # binary {0,1} masks multiplying exp'd scores.
        i = np.arange(256)[None, None, None, :]
        j = np.arange(128)[None, None, :, None]
        st = np.arange(NST)[:, None, None, None]
        r = np.arange(3)[None, :, None, None]
        qg = qs + st * 256 + i
        kg = qs + st * 256 - HALO + r * 128 + j
        allowed = (kg <= qg) & (kg > qg - WINDOW) & (kg >= 0)
        m3 = allowed.astype(np.float32)
        mask = np.empty((NST, 128, 512), dtype=np.float32)
        mask[:, :, 0:128] = m3[:, 0, :, 0:128]
        mask[:, :, 128:256] = m3[:, 2, :, 128:256]
        mask[:, :, 256:512] = m3[:, 1]
        cst_h = np.zeros((128, NST * 512 + 16 + 128), dtype=BF16)
        cst_h[:, 0:512] = mask[0]
        cst_h[:, 512:1024] = mask[1]
        cst_h[:, 1024:1040] = 1.0
        cst_h[0, 1040:1104] = 1.0    # sel row0: partitions 0..63
        cst_h[1, 1104:1168] = 1.0    # sel row1: partitions 64..127

        in_maps.append({"xt": xt_h, "wq": wq_h, "wo": wo_h, "cst": cst_h})
    return in_maps


def kernel(x, w_qkv, w_out, _trace=False, _trace_kwargs=None):
    from concourse.bass_utils import run_bass_kernel_spmd

    x = np.asarray(x, dtype=np.float32)
    w_qkv = np.asarray(w_qkv, dtype=np.float32)
    w_out = np.asarray(w_out, dtype=np.float32)
    nc = _get_nc()
    in_maps = _core_inputs(x, w_qkv, w_out)
    res = run_bass_kernel_spmd(
        nc, in_maps, list(range(NCORES)), trace=_trace, **(_trace_kwargs or {})
    )
    out = np.empty((B, S, D), dtype=np.float32)
    for c in range(NCORES):
        b, qs = c // CPB, (c % CPB) * SLOC
        o = np.asarray(res.results[c]["out"], dtype=np.float32)
        out[b, qs:qs + SLOC] = (
            o.reshape(128, NST, 8, 256).transpose(1, 3, 2, 0).reshape(SLOC, D)
        )
    if _trace:
        return out, res
    return out
